# revision 1
# baseline (speedup 1.0000x reference)
"""Trainium2 Bass kernel for the DanQ-cat-attention model.

Data-parallel over batch: 800 rows split across 8 NeuronCores (100 each),
weights replicated. BatchNorm batch stats are summed with an in-kernel
AllReduce. Everything else is core-local.

Per-core pipeline (all feature-on-partition layouts):
  A. Conv1d(4->320,k=26)+ReLU+pad+MaxPool(13) as a K=104 matmul over an
     im2col replica built by one overlapping-window DMA per batch group;
     maxpool reads conv PSUM directly via windowed reduce_max.
  B. Per-timestep single-step LSTM (both dirs) = matmul [320->480(i,g,o)]
     + fused sigmoid/tanh gating (forget gate drops out since c0=0).
  C. gene = geneexpr @ gene_w.T (K=19840 streamed); BN stats via
     ones-matmuls + AllReduce; BN+ReLU fused into the PE transpose
     eviction (per-partition scale/bias on ACT).
  D. attn[t,b] = <h[t,b,:], gene[b,:]> via broadcast-mult + ones-matmul;
     flat_T[(t,d), b] = attn * h built by strided DVE multiplies.
  E. hid = relu(flat @ lin_w.T + lin_b) with lin_w.T streamed from HBM;
     out = hid @ out_w.T + out_b reduced on DVE.
"""

import ml_dtypes
import numpy as np

import concourse.bass as bass
import concourse.mybir as mybir
import concourse.tile as tile
from concourse import bacc
from concourse.bass import AP
from concourse.bass_utils import run_bass_kernel_spmd
from concourse.masks import make_identity

F32 = mybir.dt.float32
BF16 = mybir.dt.bfloat16
# per-stage matmul dtypes (input rounding): conv, gates, gene, E-stage
DT_CONV = BF16   # conv matmul inputs (x, conv_w)
DT_GMM = BF16    # gates matmul inputs (seq, w_ih)
DT_H = F32       # gating outputs, h, geneT, attention chain
DT_GENE = BF16   # gene matmul inputs
DT_E = BF16      # flat + lin_w matmul inputs
AX = mybir.AxisListType
AF = mybir.ActivationFunctionType
MUL = mybir.AluOpType.mult
ADD = mybir.AluOpType.add
SUB = mybir.AluOpType.subtract

N_CORES = 8
B_TOTAL = 800
L_IN = 600
CIN = 4
KW = 26
KK = CIN * KW  # 104
CO = 320
T = 45
POOL = 13
NPOS = 571  # conv positions actually consumed by the pool windows
HID = 160
BN_EPS = 1e-5

KG = 19840  # 19795 padded to 155*128
KGC = KG // 128
KE = 14464  # 45*320=14400 padded to 113*128
KEC = KE // 128
H2 = 925
GB = 4  # conv batch-group size


def _groups(n, g):
    return [(i, min(g, n - i)) for i in range(0, n, g)]


def build_nc(n_cores: int, BL: int):
    """Build the per-core SPMD program. BL = batches per core."""
    BT = BL * T
    NTB = 10  # batches per N-tile in stages B/D (450 columns)
    nt_list = [(b0 * T, nb * T, b0, nb) for b0, nb in _groups(BL, NTB)]
    b_total = BL * n_cores

    nc = bacc.Bacc("TRN2", target_bir_lowering=False, debug=False,
                   num_devices=n_cores)

    # ---- I/O ----
    x_l = nc.dram_tensor("x_l", [BL, CIN, L_IN], DT_CONV, kind="ExternalInput")
    wcol = nc.dram_tensor("wcol", [KK, CO], DT_CONV, kind="ExternalInput")
    convb = nc.dram_tensor("convb", [128, 3], F32, kind="ExternalInput")
    wg = nc.dram_tensor("wg", [2, 128, 3, 480], DT_GMM, kind="ExternalInput")
    gbias = nc.dram_tensor("gbias", [2, 128, 6], F32, kind="ExternalInput")
    geT = nc.dram_tensor("geT", [128, KGC, BL], DT_GENE, kind="ExternalInput")
    gwT = nc.dram_tensor("gwT", [128, KGC, CO], DT_GENE, kind="ExternalInput")
    gbcols = nc.dram_tensor("gbcols", [128, 6], F32, kind="ExternalInput")
    linwT = nc.dram_tensor("linwT", [128, KEC, H2], DT_E, kind="ExternalInput")
    linb = nc.dram_tensor("linb", [1, H2], F32, kind="ExternalInput")
    outw = nc.dram_tensor("outw", [1, H2], F32, kind="ExternalInput")
    outb = nc.dram_tensor("outb", [1, 1], F32, kind="ExternalInput")
    y = nc.dram_tensor("y", [BL, 1], F32, kind="ExternalOutput")

    with tile.TileContext(nc) as tc:
        with (
            tc.tile_pool(name="const", bufs=1) as cst,
            tc.tile_pool(name="persist", bufs=1) as per,
            tc.tile_pool(name="dram", bufs=1, space="DRAM") as dram,
        ):
            # ---- constants ----
            wcol_sb = cst.tile([KK, CO], DT_CONV)
            nc.sync.dma_start(wcol_sb[:], wcol.ap())
            convb_sb = cst.tile([128, 3], F32)
            nc.sync.dma_start(convb_sb[:], convb.ap())
            wg_sb = cst.tile([128, 2, 3, 480], DT_GMM)
            nc.sync.dma_start(wg_sb[:, 0], wg.ap()[0])
            nc.sync.dma_start(wg_sb[:, 1], wg.ap()[1])
            gbias_sb = cst.tile([128, 2, 6], F32)
            nc.sync.dma_start(gbias_sb[:, 0], gbias.ap()[0])
            nc.sync.dma_start(gbias_sb[:, 1], gbias.ap()[1])
            gbcols_sb = cst.tile([128, 6], F32)
            nc.sync.dma_start(gbcols_sb[:], gbcols.ap())
            linb_bc = cst.tile([BL, H2], F32)
            nc.sync.dma_start(linb_bc[:], linb.ap().to_broadcast([BL, H2]))
            outw_bc = cst.tile([BL, H2], F32)
            nc.sync.dma_start(outw_bc[:], outw.ap().to_broadcast([BL, H2]))
            outb_col = cst.tile([BL, 1], F32)
            nc.sync.dma_start(outb_col[:], outb.ap().to_broadcast([BL, 1]))
            ones_k = cst.tile([128, 1], F32)
            nc.any.memset(ones_k[:], 1.0)
            ones_b = cst.tile([128, 1], DT_H)
            nc.any.memset(ones_b[:], 1.0)
            eps_c = cst.tile([128, 1], F32)
            nc.any.memset(eps_c[:], BN_EPS)
            ident = cst.tile([128, 128], F32)
            make_identity(nc, ident[:])

            # ---- persistent activations ----
            hc0 = per.tile([128, BT], DT_H)  # h dims 0..127   (fwd 0..127)
            hc1 = per.tile([128, BT], DT_H)  # h dims 128..255 (fwd 128..159 | bwd 0..95)
            hc2 = per.tile([64, BT], DT_H)   # h dims 256..319 (bwd 96..159)
            geneT = per.tile([128, 3, BL], DT_H)  # BN+ReLU'd gene, transposed
            gene_sb = per.tile([BL, CO], F32)
            gstats = per.tile([128, 6], F32)

            # gene matmul stream state (interleaved into the conv loop so the
            # in-order PE queue never stalls on gene weight DMAs)
            KB = 4  # gene k-chunks per DMA
            kb_list = list(range(0, KGC, KB))

            with (
                tc.tile_pool(name="psC1", bufs=1, space="PSUM") as psc1,
                tc.tile_pool(name="psS", bufs=1, space="PSUM") as pss,
                tc.tile_pool(name="wkc", bufs=4) as wkc,
                tc.tile_pool(name="stat", bufs=1) as stp,
            ):
              ps_gene = psc1.tile([BL, CO], F32, tag="pg")

              def emit_gene_batch(kb):
                  nkb = min(KB, KGC - kb)
                  ge_t = wkc.tile([128, KB, BL], DT_GENE, tag="ge")
                  nc.scalar.dma_start(ge_t[:, 0:nkb], geT.ap()[:, kb:kb + nkb])
                  gw_t = wkc.tile([128, KB, CO], DT_GENE, tag="gw")
                  nc.sync.dma_start(gw_t[:, 0:nkb], gwT.ap()[:, kb:kb + nkb])
                  for j in range(nkb):
                      kc = kb + j
                      nc.tensor.matmul(ps_gene[:, :], ge_t[:, j], gw_t[:, j],
                                       start=(kc == 0), stop=(kc == KGC - 1))

              # =========== Stage A: conv + maxpool (gene interleaved) =======
              with tc.tile_pool(name="seqp", bufs=1) as seqp:
                seq = seqp.tile([128, 3, BT], DT_GMM)
                with (
                  tc.tile_pool(name="wka", bufs=2) as wka,
                  tc.tile_pool(name="pooltmp", bufs=4) as ptp,
                  tc.tile_pool(name="psA", bufs=3, space="PSUM") as psa,
                ):
                  # t=0 pool window is all left-padding -> exactly 0
                  for mc in range(3):
                      mn = (128, 128, 64)[mc]
                      nc.vector.memset(
                          seq[0:mn, mc].rearrange("p (b t) -> p b t", t=T)[:, :, 0:1],
                          0.0,
                      )
                  conv_groups = _groups(BL, GB)
                  ki = 0
                  for gi, (b0, nb) in enumerate(conv_groups):
                      # ~2 gene DMA batches per conv group keeps PE fed
                      for _ in range(2):
                          if ki < len(kb_list):
                              emit_gene_batch(kb_list[ki])
                              ki += 1
                      xrep = wka.tile([KK, GB, NPOS], DT_CONV, tag="xrep")
                      for c in range(CIN):
                          src = AP(x_l.ap().tensor, (b0 * CIN + c) * L_IN,
                                   [[1, KW], [CIN * L_IN, nb], [1, NPOS]])
                          nc.gpsimd.dma_start(xrep[c * KW:(c + 1) * KW, 0:nb], src)
                      for mc in range(3):
                          m0, mn = mc * 128, (128, 128, 64)[mc]
                          for b in range(nb):
                              ps = psa.tile([128, NPOS], F32, tag="cps")
                              for p0, pn in ((0, 512), (512, NPOS - 512)):
                                  nc.tensor.matmul(
                                      ps[0:mn, p0:p0 + pn],
                                      wcol_sb[:, m0:m0 + mn],
                                      xrep[:, b, p0:p0 + pn],
                                      start=True, stop=True,
                                  )
                              tmp = ptp.tile([128, T - 1], F32, tag="ptmp")
                              nc.vector.reduce_max(
                                  tmp[0:mn, 0:1],
                                  ps[0:mn, 0:12].rearrange("p (t k) -> p t k", k=12),
                                  axis=AX.X,
                              )
                              nc.vector.reduce_max(
                                  tmp[0:mn, 1:T - 1],
                                  ps[0:mn, 12:NPOS].rearrange("p (t k) -> p t k",
                                                              k=POOL),
                                  axis=AX.X,
                              )
                              bt0 = (b0 + b) * T
                              nc.scalar.activation(
                                  seq[0:mn, mc, bt0 + 1:bt0 + T], tmp[0:mn],
                                  AF.Relu, bias=convb_sb[0:mn, mc:mc + 1],
                              )
                  while ki < len(kb_list):
                      emit_gene_batch(kb_list[ki])
                      ki += 1

                  # ---- C2a: BN stats + AllReduce trigger (hidden under B) --
                  gsq = stp.tile([BL, CO], F32)
                  nc.scalar.activation(gene_sb[:], ps_gene[:], AF.Copy)
                  nc.scalar.square(gsq[:], ps_gene[:])
                  stats = stp.tile([128, 6], F32)
                  nc.vector.memset(stats[:], 0.0)
                  for c in range(3):
                      cn = (128, 128, 64)[c]
                      ps_s = pss.tile([128, 1], F32, tag="cstat")
                      nc.tensor.matmul(ps_s[0:cn, :],
                                       gene_sb[:, c * 128:c * 128 + cn],
                                       ones_k[0:BL, :], start=True, stop=True)
                      nc.scalar.activation(stats[0:cn, c:c + 1], ps_s[0:cn, :],
                                           AF.Copy)
                      ps_q = pss.tile([128, 1], F32, tag="cstat")
                      nc.tensor.matmul(ps_q[0:cn, :],
                                       gsq[:, c * 128:c * 128 + cn],
                                       ones_k[0:BL, :], start=True, stop=True)
                      nc.scalar.activation(stats[0:cn, 3 + c:4 + c], ps_q[0:cn, :],
                                           AF.Copy)
                  cc_in = dram.tile([128, 6], F32)
                  cc_out = dram.tile([128, 6], F32)
                  nc.scalar.dma_start(cc_in[:], stats[:])
                  nc.gpsimd.collective_compute(
                      "AllReduce", ADD,
                      replica_groups=[list(range(n_cores))],
                      ins=[cc_in.opt()], outs=[cc_out.opt()],
                  )
                  nc.gpsimd.dma_start(gstats[:], cc_out[:])

                # ---- Stage B: gates + gating ----
                with (
                    tc.tile_pool(name="wkb", bufs=3) as wkb,
                    tc.tile_pool(name="psB", bufs=3, space="PSUM") as psb,
                ):
                    for d in range(2):
                        for n0, nn, _, _ in nt_list:
                            ps_i = psb.tile([128, 450], F32, tag="g128")
                            ps_g = psb.tile([128, 450], F32, tag="g128")
                            ps_o = psb.tile([128, 450], F32, tag="g128")
                            ps3i = psb.tile([32, 450], F32, tag="g32")
                            ps3g = psb.tile([32, 450], F32, tag="g32")
                            ps3o = psb.tile([32, 450], F32, tag="g32")
                            for mc, (m0, mn, pst) in enumerate(
                                ((0, 128, ps_i), (128, 128, ps_g),
                                 (256, 128, ps_o), (384, 32, ps3i),
                                 (416, 32, ps3g), (448, 32, ps3o))
                            ):
                                for kc in range(3):
                                    kn = (128, 128, 64)[kc]
                                    nc.tensor.matmul(
                                        pst[0:mn, 0:nn],
                                        wg_sb[0:kn, d, kc, m0:m0 + mn],
                                        seq[0:kn, kc, n0:n0 + nn],
                                        start=(kc == 0), stop=(kc == 2),
                                    )
                            bia = gbias_sb[:, d]
                            si = wkb.tile([128, 450], DT_H, tag="si")
                            tg = wkb.tile([128, 450], DT_H, tag="tg")
                            so = wkb.tile([128, 450], DT_H, tag="so")
                            tc_ = wkb.tile([128, 450], DT_H, tag="tc")
                            nc.scalar.activation(si[:, 0:nn], ps_i[:, 0:nn],
                                                 AF.Sigmoid, bias=bia[:, 0:1])
                            nc.scalar.activation(tg[:, 0:nn], ps_g[:, 0:nn],
                                                 AF.Tanh, bias=bia[:, 1:2])
                            nc.scalar.activation(so[:, 0:nn], ps_o[:, 0:nn],
                                                 AF.Sigmoid, bias=bia[:, 2:3])
                            nc.vector.tensor_tensor(tc_[:, 0:nn], si[:, 0:nn],
                                                    tg[:, 0:nn], MUL)
                            nc.scalar.activation(tc_[:, 0:nn], tc_[:, 0:nn],
                                                 AF.Tanh)
                            # 32-row leftovers i'/g'/o', each base-0
                            s3i = wkb.tile([32, 450], DT_H, tag="s3i")
                            s3g = wkb.tile([32, 450], DT_H, tag="s3g")
                            s3o = wkb.tile([32, 450], DT_H, tag="s3o")
                            nc.scalar.activation(s3i[:, 0:nn], ps3i[:, 0:nn],
                                                 AF.Sigmoid, bias=bia[0:32, 3:4])
                            nc.scalar.activation(s3g[:, 0:nn], ps3g[:, 0:nn],
                                                 AF.Tanh, bias=bia[0:32, 4:5])
                            nc.scalar.activation(s3o[:, 0:nn], ps3o[:, 0:nn],
                                                 AF.Sigmoid, bias=bia[0:32, 5:6])
                            t3 = wkb.tile([32, 450], DT_H, tag="t3")
                            nc.vector.tensor_tensor(t3[:, 0:nn], s3i[:, 0:nn],
                                                    s3g[:, 0:nn], MUL)
                            nc.scalar.activation(t3[:, 0:nn], t3[:, 0:nn], AF.Tanh)
                            if d == 0:
                                nc.vector.tensor_tensor(
                                    hc0[:, n0:n0 + nn], tc_[:, 0:nn], so[:, 0:nn],
                                    MUL)
                                nc.vector.tensor_tensor(
                                    hc1[0:32, n0:n0 + nn], t3[:, 0:nn],
                                    s3o[:, 0:nn], MUL)
                            else:
                                # quadrant rule: split the base-32 96-row write
                                for q in range(3):
                                    nc.vector.tensor_tensor(
                                        hc1[32 + 32 * q:64 + 32 * q, n0:n0 + nn],
                                        tc_[32 * q:32 * q + 32, 0:nn],
                                        so[32 * q:32 * q + 32, 0:nn], MUL)
                                nc.vector.tensor_tensor(
                                    hc2[0:32, n0:n0 + nn], tc_[96:128, 0:nn],
                                    so[96:128, 0:nn], MUL)
                                nc.vector.tensor_tensor(
                                    hc2[32:64, n0:n0 + nn], t3[:, 0:nn],
                                    s3o[:, 0:nn], MUL)

              # ---- C2b: BN scale/bias + fused transpose (AR already done) ----
              mean = stp.tile([128, 3], F32)
              var = stp.tile([128, 3], F32)
              scl = stp.tile([128, 3], F32)
              nbi = stp.tile([128, 3], F32)
              inv_b = 1.0 / float(b_total)
              nc.scalar.activation(mean[:], gstats[:, 0:3], AF.Copy, scale=inv_b)
              nc.scalar.activation(var[:], gstats[:, 3:6], AF.Copy, scale=inv_b)
              msq = stp.tile([128, 3], F32)
              nc.scalar.square(msq[:], mean[:])
              nc.vector.tensor_tensor(var[:], var[:], msq[:], SUB)
              std = stp.tile([128, 3], F32)
              nc.scalar.activation(std[:], var[:], AF.Sqrt, bias=eps_c[:])
              rstd = stp.tile([128, 3], F32)
              nc.vector.reciprocal(rstd[:], std[:])
              nc.vector.tensor_tensor(scl[:], gbcols_sb[:, 0:3], rstd[:], MUL)
              nc.vector.tensor_tensor(nbi[:], mean[:], scl[:], MUL)
              nc.vector.tensor_tensor(nbi[:], gbcols_sb[:, 3:6], nbi[:], SUB)
              for c in range(3):
                  cn = (128, 128, 64)[c]
                  ps_t = pss.tile([128, BL], F32, tag="cstat")
                  nc.tensor.transpose(ps_t[0:cn, :],
                                      gene_sb[:, c * 128:c * 128 + cn],
                                      ident[0:BL, 0:BL])
                  nc.scalar.activation(geneT[0:cn, c, :], ps_t[0:cn, :], AF.Relu,
                                       bias=nbi[0:cn, c:c + 1],
                                       scale=scl[0:cn, c:c + 1])

            # =========== Stage D: attention + flat_T ===========
            with tc.tile_pool(name="late", bufs=1) as late:
              attn_bc = late.tile([128, BT], DT_H)
              flatT = late.tile([128, KEC, BL], DT_E)
              with (
                tc.tile_pool(name="wkd", bufs=3) as wkd,
                tc.tile_pool(name="psD", bufs=4, space="PSUM") as psd,
              ):
                  for n0, nn, b0, nb in nt_list:
                      prod = wkd.tile([128, 3, 450], DT_H, tag="prod")
                      for c, (htile, cn) in enumerate(((hc0, 128), (hc1, 128),
                                                       (hc2, 64))):
                          nc.vector.tensor_tensor(
                              prod[0:cn, c, 0:nn].rearrange("p (b t) -> p b t", t=T),
                              htile[0:cn, n0:n0 + nn].rearrange("p (b t) -> p b t",
                                                                t=T),
                              geneT[0:cn, c, b0:b0 + nb][:, :, None]
                              .to_broadcast([cn, nb, T]),
                              MUL,
                          )
                      ps_a = psd.tile([1, 450], F32, tag="ps_a")
                      for c in range(3):
                          cn = (128, 128, 64)[c]
                          nc.tensor.matmul(ps_a[0:1, 0:nn], ones_b[0:cn, :],
                                           prod[0:cn, c, 0:nn],
                                           start=(c == 0), stop=(c == 2))
                      attn_row = wkd.tile([1, 450], DT_H, tag="arow")
                      nc.scalar.activation(attn_row[:, 0:nn], ps_a[:, 0:nn], AF.Copy)
                      nc.gpsimd.partition_broadcast(attn_bc[:, n0:n0 + nn],
                                                    attn_row[:, 0:nn])
                  # flat_T[(t,d), b] = attn[b,t] * h[d, (b,t)]
                  nc.vector.memset(flatT[64:128, KEC - 1, :], 0.0)
                  for t in range(T):
                      r0 = t * CO
                      for htile, cn, dglob in ((hc0, 128, 0), (hc1, 128, 128),
                                               (hc2, 64, 256)):
                          # split [dglob, dglob+cn) at dest 128-row boundaries
                          s = 0
                          while s < cn:
                              r = r0 + dglob + s
                              kc, p = divmod(r, 128)
                              ln = min(cn - s, 128 - p)
                              nc.vector.tensor_tensor(
                                  flatT[p:p + ln, kc, :],
                                  htile[s + 0:s + ln, t::T],
                                  attn_bc[s:s + ln, t::T],
                                  MUL,
                              )
                              s += ln

              # =========== Stage E: hid = relu(flat @ lin_w.T + b); out =====
              with (
                  tc.tile_pool(name="wke", bufs=3) as wke,
                  tc.tile_pool(name="psE", bufs=1, space="PSUM") as pse,
              ):
                  ps_hid = pse.tile([BL, H2], F32, tag="ph")
                  EB = 4  # lin k-chunks per DMA
                  for ki, kb in enumerate(range(0, KEC, EB)):
                      nkb = min(EB, KEC - kb)
                      lw_t = wke.tile([128, EB, H2], DT_E, tag="lw")
                      eng = nc.sync if ki % 2 == 0 else nc.scalar
                      eng.dma_start(lw_t[:, 0:nkb], linwT.ap()[:, kb:kb + nkb])
                      for j in range(nkb):
                          kc = kb + j
                          for j0, jn in ((0, 512), (512, H2 - 512)):
                              nc.tensor.matmul(ps_hid[:, j0:j0 + jn],
                                               flatT[:, kc, :],
                                               lw_t[:, j, j0:j0 + jn],
                                               start=(kc == 0),
                                               stop=(kc == KEC - 1))
                  hid = wke.tile([BL, H2], F32, tag="hid")
                  nc.vector.tensor_tensor(hid[:], ps_hid[:], linb_bc[:], ADD)
                  nc.scalar.activation(hid[:], hid[:], AF.Relu)
                  hw = wke.tile([BL, H2], F32, tag="hw")
                  nc.vector.tensor_tensor(hw[:], hid[:], outw_bc[:], MUL)
                  y_sb = wke.tile([BL, 1], F32, tag="ysb")
                  nc.vector.reduce_sum(y_sb[:], hw[:], axis=AX.X)
                  nc.vector.tensor_tensor(y_sb[:], y_sb[:], outb_col[:], ADD)
                  nc.sync.dma_start(y.ap(), y_sb[:])

    nc.compile()
    return nc


def make_in_maps(inputs, n_cores: int, BL: int):
    """Host-side prep: shard + transpose + pad + reorder weights."""
    f32 = np.float32

    def np_dt(dt):
        return ml_dtypes.bfloat16 if dt == BF16 else np.float32

    d_conv, d_gate, d_gene, d_e = (np_dt(DT_CONV), np_dt(DT_GMM),
                                   np_dt(DT_GENE), np_dt(DT_E))

    def pad_rows(a, n):
        return np.pad(a, ((0, n - a.shape[0]),) + ((0, 0),) * (a.ndim - 1))

    conv_w = np.asarray(inputs["conv_w"], f32)
    wcol = np.ascontiguousarray(conv_w.transpose(1, 2, 0).reshape(KK, CO)).astype(d_conv)
    convb = np.ascontiguousarray(
        pad_rows(np.asarray(inputs["conv_b"], f32), 384).reshape(3, 128).T)

    def gate_prep(w_ih, b_ih, b_hh):
        W = np.asarray(w_ih, f32).T  # [320, 640]
        b = (np.asarray(b_ih, f32) + np.asarray(b_hh, f32))  # [640]
        cols = (list(range(0, 128)) + list(range(320, 448)) +
                list(range(480, 608)) + list(range(128, 160)) +
                list(range(448, 480)) + list(range(608, 640)))
        Wr = W[:, cols]  # [320, 480]
        br = b[cols]  # [480]
        Wr = pad_rows(Wr, 384).reshape(3, 128, 480).transpose(1, 0, 2).astype(d_gate)
        # cols: i(0:128), g(0:128), o(0:128), then i'/g'/o' each at rows 0:32
        bc = np.zeros((128, 6), np.float32)
        bc[:, 0] = br[0:128]
        bc[:, 1] = br[128:256]
        bc[:, 2] = br[256:384]
        bc[0:32, 3] = br[384:416]
        bc[0:32, 4] = br[416:448]
        bc[0:32, 5] = br[448:480]
        br = bc
        return np.ascontiguousarray(Wr), np.ascontiguousarray(br)

    wgf, bgf = gate_prep(inputs["w_ih_f"], inputs["b_ih_f"], inputs["b_hh_f"])
    wgb, bgb = gate_prep(inputs["w_ih_b"], inputs["b_ih_b"], inputs["b_hh_b"])
    wg = np.stack([wgf, wgb])  # [2, 128, 3, 480]
    gbias = np.stack([bgf, bgb])  # [2, 128, 4]

    gene_w = np.asarray(inputs["gene_w"], f32)  # [320, 19795]
    gwT = np.ascontiguousarray(pad_rows(np.ascontiguousarray(gene_w.T), KG)
                               .reshape(KGC, 128, CO).transpose(1, 0, 2)).astype(d_gene)
    gamma = pad_rows(np.asarray(inputs["bn_gamma"], f32), 384).reshape(3, 128).T
    beta = pad_rows(np.asarray(inputs["bn_beta"], f32), 384).reshape(3, 128).T
    gbcols = np.ascontiguousarray(np.concatenate([gamma, beta], 1))  # [128, 6]

    lin_w = np.asarray(inputs["lin_w"], f32)  # [925, 14400]
    linwT = np.ascontiguousarray(pad_rows(np.ascontiguousarray(lin_w.T), KE)
                                 .reshape(KEC, 128, H2).transpose(1, 0, 2)).astype(d_e)
    linb = np.asarray(inputs["lin_b"], f32).reshape(1, H2)
    outw = np.asarray(inputs["out_w"], f32).reshape(1, H2)
    outb = np.asarray(inputs["out_b"], f32).reshape(1, 1)

    x = np.asarray(inputs["x"], f32)
    ge = np.asarray(inputs["geneexpr"], f32)

    shared = dict(wcol=wcol, convb=convb, wg=wg, gbias=gbias, gwT=gwT,
                  gbcols=gbcols, linwT=linwT, linb=linb, outw=outw, outb=outb)
    in_maps = []
    for i in range(n_cores):
        sl = slice(i * BL, (i + 1) * BL)
        geT = np.ascontiguousarray(pad_rows(np.ascontiguousarray(ge[sl].T), KG)
                                   .reshape(KGC, 128, BL).transpose(1, 0, 2)).astype(d_gene)
        m = dict(shared)
        m["x_l"] = np.ascontiguousarray(x[sl]).astype(d_conv)
        m["geT"] = geT
        in_maps.append(m)
    return in_maps


_NC_CACHE = {}


def _get_nc(n_cores, BL):
    key = (n_cores, BL)
    if key not in _NC_CACHE:
        _NC_CACHE[key] = build_nc(n_cores, BL)
    return _NC_CACHE[key]


def kernel(**inputs) -> np.ndarray:
    BL = B_TOTAL // N_CORES
    nc = _get_nc(N_CORES, BL)
    in_maps = make_in_maps(inputs, N_CORES, BL)
    res = run_bass_kernel_spmd(nc, in_maps, list(range(N_CORES)))
    return np.concatenate([res.results[i]["y"] for i in range(N_CORES)], axis=0)



# revision 8
# speedup vs baseline: 1.1638x; 1.1638x over previous
"""Trainium2 Bass kernel for the DanQ-cat-attention model.

Data-parallel over batch: 800 rows split across 8 NeuronCores (100 each),
weights replicated. BatchNorm batch stats are summed with an in-kernel
AllReduce. Everything else is core-local.

All [*, BT] activations use a t-major column layout: col = t*BL + b. This
makes the stage-E lhsT slices (per-timestep [d, b] panels) contiguous.

Per-core pipeline:
  A. Conv1d(4->320,k=26)+ReLU+pad+MaxPool(13) as a K=104 matmul over an
     im2col replica; maxpool splits between two routes to balance DVE/ACT:
     R1: DVE reduce_max straight from PSUM f32, ACT relu+bias on 44 cols.
     R2: ACT relu+bias PSUM->SBUF bf16 full width, DVE reduces bf16 (2x).
     gene = geneexpr @ gene_w.T matmuls stream interleaved with conv.
  B. Per-timestep single-step LSTM (both dirs): gates packed in 9 aligned
     m-chunks (i|g|o blocks of 320 rows each), m-chunk-major over 9 n-tiles
     of 500 with a double-buffered single PSUM bank; sigma/tanh staged to
     SBUF bf16 full width; gating = 3 DVE mults + 3 ACT tanh + 3 DVE mults,
     all partition-aligned.
  C. BN stats via ones-matmuls + AllReduce (fired before B, lands under B);
     BN+ReLU fused into the PE transpose eviction -> geneT bf16.
  D/E fused per 5-timestep block: prod=h*geneT -> ones-matmul attn ->
     partition_broadcast -> ha = h*attn (bf16); hid += ha_t.T @ lin_w_t
     accumulated over 45x3 chunks with double-buffered weight streaming;
     out = relu(hid+b) @ out_w reduced on DVE.
"""

import ml_dtypes
import numpy as np

import concourse.bass as bass
import concourse.mybir as mybir
import concourse.tile as tile
from concourse import bacc
from concourse.bass import AP
from concourse.bass_utils import run_bass_kernel_spmd
from concourse.masks import make_identity

F32 = mybir.dt.float32
BF16 = mybir.dt.bfloat16
AX = mybir.AxisListType
AF = mybir.ActivationFunctionType
MUL = mybir.AluOpType.mult
ADD = mybir.AluOpType.add
SUB = mybir.AluOpType.subtract

N_CORES = 8
B_TOTAL = 800
L_IN = 600
CIN = 4
KW = 26
KK = CIN * KW  # 104
CO = 320
T = 45
POOL = 13
NPOS = 571  # conv positions consumed by the pool windows
HID = 160
BN_EPS = 1e-5

KG = 19840  # 19795 padded to 155*128
KGC = KG // 128
H2 = 925
GB = 2  # conv batch-group size

# stage B chunk geometry: i|g|o blocks of 320 rows -> 9 m-chunks
MK_OFF = (0, 128, 256, 320, 448, 576, 640, 768, 896)
MK_W = (128, 128, 64, 128, 128, 64, 128, 128, 64)
MK_AF = (AF.Sigmoid, AF.Sigmoid, AF.Sigmoid,
         AF.Tanh, AF.Tanh, AF.Tanh,
         AF.Sigmoid, AF.Sigmoid, AF.Sigmoid)
NT = 9       # n-tiles of 500 cols (BT = 4500)
NTW = 500
TBLK = 5     # timesteps per D/E block
NBLK = T // TBLK  # 9


def build_nc(n_cores: int, BL: int):
    BT = BL * T
    b_total = BL * n_cores

    nc = bacc.Bacc("TRN2", target_bir_lowering=False, debug=False,
                   num_devices=n_cores)

    # ---- I/O ----
    x_l = nc.dram_tensor("x_l", [BL, CIN, L_IN], BF16, kind="ExternalInput")
    wcol = nc.dram_tensor("wcol", [KK, CO], BF16, kind="ExternalInput")
    convb = nc.dram_tensor("convb", [128, 3], F32, kind="ExternalInput")
    wg2 = nc.dram_tensor("wg2", [128, 3, 960], BF16, kind="ExternalInput")
    gbias2 = nc.dram_tensor("gbias2", [128, 9], F32, kind="ExternalInput")
    geT = nc.dram_tensor("geT", [128, KGC, BL], BF16, kind="ExternalInput")
    gwT = nc.dram_tensor("gwT", [128, KGC, CO], BF16, kind="ExternalInput")
    gbcols = nc.dram_tensor("gbcols", [128, 6], F32, kind="ExternalInput")
    lw0 = nc.dram_tensor("lw0", [128, T, H2], BF16, kind="ExternalInput")
    lw1 = nc.dram_tensor("lw1", [128, T, H2], BF16, kind="ExternalInput")
    lw2 = nc.dram_tensor("lw2", [64, T, H2], BF16, kind="ExternalInput")
    linb = nc.dram_tensor("linb", [1, H2], F32, kind="ExternalInput")
    outw = nc.dram_tensor("outw", [1, H2], F32, kind="ExternalInput")
    outb = nc.dram_tensor("outb", [1, 1], F32, kind="ExternalInput")
    y = nc.dram_tensor("y", [BL, 1], F32, kind="ExternalOutput")

    lw_dram = (lw0, lw1, lw2)

    with tile.TileContext(nc) as tc:
        with (
            tc.tile_pool(name="const", bufs=1) as cst,
            tc.tile_pool(name="persist", bufs=1) as per,
            tc.tile_pool(name="dram", bufs=1, space="DRAM") as dram,
            tc.tile_pool(name="lwp", bufs=2) as lwp,
        ):
            # ---- constants ----
            wcol_sb = cst.tile([KK, CO], BF16)
            nc.sync.dma_start(wcol_sb[:], wcol.ap())
            convb_sb = cst.tile([128, 3], F32)
            nc.sync.dma_start(convb_sb[:], convb.ap())
            wg_sb = cst.tile([128, 3, 960], BF16)
            nc.sync.dma_start(wg_sb[:], wg2.ap())
            gbias_sb = cst.tile([128, 9], F32)
            nc.sync.dma_start(gbias_sb[:], gbias2.ap())
            gbcols_sb = cst.tile([128, 6], F32)
            nc.sync.dma_start(gbcols_sb[:], gbcols.ap())
            linb_bc = cst.tile([BL, H2], F32)
            nc.sync.dma_start(linb_bc[:], linb.ap().to_broadcast([BL, H2]))
            outw_bc = cst.tile([BL, H2], F32)
            nc.sync.dma_start(outw_bc[:], outw.ap().to_broadcast([BL, H2]))
            outb_col = cst.tile([BL, 1], F32)
            nc.sync.dma_start(outb_col[:], outb.ap().to_broadcast([BL, 1]))
            ones_k = cst.tile([128, 1], F32)
            nc.any.memset(ones_k[:], 1.0)
            ones_b = cst.tile([128, 1], BF16)
            nc.any.memset(ones_b[:], 1.0)
            eps_c = cst.tile([128, 1], F32)
            nc.any.memset(eps_c[:], BN_EPS)
            ident = cst.tile([128, 128], F32)
            make_identity(nc, ident[:])

            # ---- persistent activations ----
            hc0 = per.tile([128, BT], BF16)   # h dims 0..127
            hc1 = per.tile([128, BT], BF16)   # h dims 128..255
            hc2 = per.tile([64, BT], BF16)    # h dims 256..319
            geneT = per.tile([128, 3, BL], BF16)  # BN+ReLU'd gene, transposed
            gene_sb = per.tile([BL, CO], F32)
            gstats = per.tile([128, 6], F32)

            # E-weight prefetch: first block per chunk starts loading now
            lw_tiles = {}

            def lw_fetch(blk, c, eng):
                cn = (128, 128, 64)[c]
                t0 = blk * TBLK
                tl = lwp.tile([cn, TBLK, H2], BF16, tag=f"lw{c}")
                eng.dma_start(tl[:], lw_dram[c].ap()[:, t0:t0 + TBLK])
                lw_tiles[(blk, c)] = tl

            for blk0 in (0, 1):
                for c in range(3):
                    lw_fetch(blk0, c, nc.scalar if c % 2 else nc.sync)

            # gene matmul stream state
            KB = 4  # gene k-chunks per DMA
            kb_list = list(range(0, KGC, KB))

            with (
                tc.tile_pool(name="psC1", bufs=1, space="PSUM") as psc1,
                tc.tile_pool(name="psS", bufs=1, space="PSUM") as pss,
                tc.tile_pool(name="stat", bufs=1) as stp,
            ):
              ps_gene = psc1.tile([BL, CO], F32, tag="pg")

              # =========== Stage A: conv + maxpool (gene interleaved) =======
              with tc.tile_pool(name="seqp", bufs=1) as seqp:
                seq = seqp.tile([128, 3, BT], BF16)
                with (
                  tc.tile_pool(name="wka", bufs=2) as wka,
                  tc.tile_pool(name="wkc", bufs=4) as wkc,
                  tc.tile_pool(name="pooltmp", bufs=4) as ptp,
                  tc.tile_pool(name="psA", bufs=3, space="PSUM") as psa,
                ):
                  def emit_gene_batch(kb):
                      nkb = min(KB, KGC - kb)
                      ge_t = wkc.tile([128, KB, BL], BF16, tag="ge")
                      nc.scalar.dma_start(ge_t[:, 0:nkb],
                                          geT.ap()[:, kb:kb + nkb])
                      gw_t = wkc.tile([128, KB, CO], BF16, tag="gw")
                      nc.sync.dma_start(gw_t[:, 0:nkb],
                                        gwT.ap()[:, kb:kb + nkb])
                      for j in range(nkb):
                          kc = kb + j
                          nc.tensor.matmul(ps_gene[:, :], ge_t[:, j],
                                           gw_t[:, j], start=(kc == 0),
                                           stop=(kc == KGC - 1))
                  # t=0 pool window is all left-padding -> exactly 0
                  for mc in range(3):
                      mn = (128, 128, 64)[mc]
                      nc.vector.memset(seq[0:mn, mc, 0:BL], 0.0)
                  n_groups = BL // GB
                  ki = 0
                  for gi in range(n_groups):
                      b0 = gi * GB
                      # keep the gene stream ahead so AR can fire early
                      quota = 4 if gi < 5 else 3
                      for _ in range(quota):
                          if ki < len(kb_list):
                              emit_gene_batch(kb_list[ki])
                              ki += 1
                      xrep = wka.tile([KK, GB, NPOS], BF16, tag="xrep")
                      for c in range(CIN):
                          src = AP(x_l.ap().tensor, (b0 * CIN + c) * L_IN,
                                   [[1, KW], [CIN * L_IN, GB], [1, NPOS]])
                          eng = nc.gpsimd if c % 2 == 0 else nc.sync
                          eng.dma_start(xrep[c * KW:(c + 1) * KW, :], src)
                      for mc in range(3):
                          m0, mn = mc * 128, (128, 128, 64)[mc]
                          for b in range(GB):
                              bb = b0 + b
                              ps = psa.tile([128, NPOS], F32, tag="cps")
                              for p0, pn in ((0, 512), (512, NPOS - 512)):
                                  nc.tensor.matmul(
                                      ps[0:mn, p0:p0 + pn],
                                      wcol_sb[:, m0:m0 + mn],
                                      xrep[:, b, p0:p0 + pn],
                                      start=True, stop=True,
                                  )
                              # seq dst views (t-major): col = t*BL + bb
                              seq_mc = seq[0:mn, mc].rearrange(
                                  "p (t b) -> p t b", b=BL)
                              if bb % 10 < 7:
                                  # R2: ACT relu+bias full width -> bf16,
                                  # then cheap bf16 reduces
                                  st = wka.tile([128, NPOS], BF16, tag="st")
                                  nc.scalar.activation(
                                      st[0:mn, :], ps[0:mn, :], AF.Relu,
                                      bias=convb_sb[0:mn, mc:mc + 1])
                                  nc.vector.reduce_max(
                                      seq_mc[:, 1:2, bb], st[0:mn, 0:12],
                                      axis=AX.X)
                                  nc.vector.reduce_max(
                                      seq_mc[:, 2:T, bb],
                                      st[0:mn, 12:NPOS].rearrange(
                                          "p (t k) -> p t k", k=POOL),
                                      axis=AX.X)
                              else:
                                  # R1: DVE reduces from PSUM f32,
                                  # ACT relu+bias on the 44 pooled cols
                                  tmp = ptp.tile([128, T - 1], F32, tag="ptmp")
                                  nc.vector.reduce_max(
                                      tmp[0:mn, 0:1], ps[0:mn, 0:12],
                                      axis=AX.X)
                                  nc.vector.reduce_max(
                                      tmp[0:mn, 1:T - 1],
                                      ps[0:mn, 12:NPOS].rearrange(
                                          "p (t k) -> p t k", k=POOL),
                                      axis=AX.X)
                                  nc.scalar.activation(
                                      seq_mc[:, 1:T, bb], tmp[0:mn],
                                      AF.Relu, bias=convb_sb[0:mn, mc:mc + 1])
                  while ki < len(kb_list):
                      emit_gene_batch(kb_list[ki])
                      ki += 1

                  # ---- C2a: BN stats + AllReduce trigger (hidden under B) --
                  gsq = stp.tile([BL, CO], F32)
                  nc.scalar.activation(gene_sb[:], ps_gene[:], AF.Copy)
                  nc.scalar.square(gsq[:], ps_gene[:])
                  stats = stp.tile([128, 6], F32)
                  nc.vector.memset(stats[:], 0.0)
                  for c in range(3):
                      cn = (128, 128, 64)[c]
                      ps_s = pss.tile([128, 1], F32, tag="cstat")
                      nc.tensor.matmul(ps_s[0:cn, :],
                                       gene_sb[:, c * 128:c * 128 + cn],
                                       ones_k[0:BL, :], start=True, stop=True)
                      nc.scalar.activation(stats[0:cn, c:c + 1], ps_s[0:cn, :],
                                           AF.Copy)
                      ps_q = pss.tile([128, 1], F32, tag="cstat")
                      nc.tensor.matmul(ps_q[0:cn, :],
                                       gsq[:, c * 128:c * 128 + cn],
                                       ones_k[0:BL, :], start=True, stop=True)
                      nc.scalar.activation(stats[0:cn, 3 + c:4 + c],
                                           ps_q[0:cn, :], AF.Copy)
                  cc_in = dram.tile([128, 6], F32)
                  cc_out = dram.tile([128, 6], F32)
                  nc.scalar.dma_start(cc_in[:], stats[:])
                  nc.gpsimd.collective_compute(
                      "AllReduce", ADD,
                      replica_groups=[list(range(n_cores))],
                      ins=[cc_in.opt()], outs=[cc_out.opt()],
                  )
                  nc.gpsimd.dma_start(gstats[:], cc_out[:])

                # ---- Stage B: gates, m-chunk-major, staged to SBUF ----
                with (
                    tc.tile_pool(name="stg", bufs=1) as stg,
                    tc.tile_pool(name="psB", bufs=2, space="PSUM") as psb,
                ):
                    stage = []
                    for mk in range(9):
                        stage.append(stg.tile([MK_W[mk], BT], BF16,
                                              name=f"stage{mk}"))
                    for mk in range(9):
                        m0, mn = MK_OFF[mk], MK_W[mk]
                        for nt in range(NT):
                            n0 = nt * NTW
                            ps = psb.tile([mn, NTW], F32, tag=f"g{mn}")
                            for kc in range(3):
                                kn = (128, 128, 64)[kc]
                                nc.tensor.matmul(
                                    ps[:, :],
                                    wg_sb[0:kn, kc, m0:m0 + mn],
                                    seq[0:kn, kc, n0:n0 + NTW],
                                    start=(kc == 0), stop=(kc == 2),
                                )
                            nc.scalar.activation(
                                stage[mk][:, n0:n0 + NTW], ps[:, :],
                                MK_AF[mk], bias=gbias_sb[0:mn, mk:mk + 1])
                    # gating: c = sig(i)*tanh(g); h = sig(o)*tanh(c)
                    # chunk-aligned: i=(0,1,2), g=(3,4,5), o=(6,7,8);
                    # tanh(c) overwrites the g-chunk stage tiles in place
                    for j in range(3):
                        nc.vector.tensor_tensor(stage[3 + j][:], stage[j][:],
                                                stage[3 + j][:], MUL)
                        nc.scalar.activation(stage[3 + j][:], stage[3 + j][:],
                                             AF.Tanh)
                    for j, hdst in enumerate((hc0, hc1, hc2)):
                        nc.vector.tensor_tensor(hdst[:], stage[6 + j][:],
                                                stage[3 + j][:], MUL)

              # ---- C2b: BN scale/bias + fused transpose (AR already done) --
              mean = stp.tile([128, 3], F32)
              var = stp.tile([128, 3], F32)
              scl = stp.tile([128, 3], F32)
              nbi = stp.tile([128, 3], F32)
              inv_b = 1.0 / float(b_total)
              nc.scalar.activation(mean[:], gstats[:, 0:3], AF.Copy, scale=inv_b)
              nc.scalar.activation(var[:], gstats[:, 3:6], AF.Copy, scale=inv_b)
              msq = stp.tile([128, 3], F32)
              nc.scalar.square(msq[:], mean[:])
              nc.vector.tensor_tensor(var[:], var[:], msq[:], SUB)
              std = stp.tile([128, 3], F32)
              nc.scalar.activation(std[:], var[:], AF.Sqrt, bias=eps_c[:])
              rstd = stp.tile([128, 3], F32)
              nc.vector.reciprocal(rstd[:], std[:])
              nc.vector.tensor_tensor(scl[:], gbcols_sb[:, 0:3], rstd[:], MUL)
              nc.vector.tensor_tensor(nbi[:], mean[:], scl[:], MUL)
              nc.vector.tensor_tensor(nbi[:], gbcols_sb[:, 3:6], nbi[:], SUB)
              for c in range(3):
                  cn = (128, 128, 64)[c]
                  ps_t = pss.tile([128, BL], F32, tag="cstat")
                  nc.tensor.transpose(ps_t[0:cn, :],
                                      gene_sb[:, c * 128:c * 128 + cn],
                                      ident[0:BL, 0:BL])
                  nc.scalar.activation(geneT[0:cn, c, :], ps_t[0:cn, :], AF.Relu,
                                       bias=nbi[0:cn, c:c + 1],
                                       scale=scl[0:cn, c:c + 1])

            # =========== Stage D+E fused, per 5-timestep block ===========
            with (
                tc.tile_pool(name="wkd", bufs=3) as wkd,
                tc.tile_pool(name="psD", bufs=2, space="PSUM") as psd,
                tc.tile_pool(name="psE", bufs=1, space="PSUM") as pse,
            ):
                hcs = (hc0, hc1, hc2)
                ps_hid = pse.tile([BL, H2], F32, tag="ph")
                for blk in range(NBLK):
                    n0 = blk * TBLK * BL
                    # attention dot: prod = h * geneT(broadcast over t)
                    prod = wkd.tile([128, 3, NTW], BF16, tag="prod")
                    for c in range(3):
                        cn = (128, 128, 64)[c]
                        nc.vector.tensor_tensor(
                            prod[0:cn, c].rearrange("p (t b) -> p t b", b=BL),
                            hcs[c][0:cn, n0:n0 + NTW].rearrange(
                                "p (t b) -> p t b", b=BL),
                            geneT[0:cn, c, :][:, None, :]
                            .to_broadcast([cn, TBLK, BL]),
                            MUL,
                        )
                    ps_a = psd.tile([1, NTW], F32, tag="ps_a")
                    for c in range(3):
                        cn = (128, 128, 64)[c]
                        nc.tensor.matmul(ps_a[0:1, :], ones_b[0:cn, :],
                                         prod[0:cn, c, :],
                                         start=(c == 0), stop=(c == 2))
                    attn_row = wkd.tile([1, NTW], BF16, tag="arow")
                    nc.scalar.activation(attn_row[:], ps_a[:], AF.Copy)
                    attn_bc = wkd.tile([128, NTW], BF16, tag="abc")
                    nc.gpsimd.partition_broadcast(attn_bc[:], attn_row[:])
                    # ha = h * attn; E matmuls for this block
                    ha = wkd.tile([128, 3, NTW], BF16, tag="ha")
                    for c in range(3):
                        cn = (128, 128, 64)[c]
                        nc.vector.tensor_tensor(ha[0:cn, c],
                                                hcs[c][0:cn, n0:n0 + NTW],
                                                attn_bc[0:cn, :], MUL)
                    for tt in range(TBLK):
                        tg = blk * TBLK + tt
                        for c in range(3):
                            cn = (128, 128, 64)[c]
                            lwt = lw_tiles[(blk, c)]
                            for j0, jn in ((0, 512), (512, H2 - 512)):
                                nc.tensor.matmul(
                                    ps_hid[:, j0:j0 + jn],
                                    ha[0:cn, c, tt * BL:(tt + 1) * BL],
                                    lwt[:, tt, j0:j0 + jn],
                                    start=(tg == 0 and c == 0),
                                    stop=(tg == T - 1 and c == 2),
                                )
                    # prefetch block blk+2 into the buffer blk just freed
                    if blk + 2 < NBLK:
                        for c in range(3):
                            del lw_tiles[(blk, c)]
                            lw_fetch(blk + 2, c,
                                     nc.scalar if c % 2 else nc.sync)

                # ---- E epilogue ----
                hid = wkd.tile([BL, H2], F32, tag="hid")
                nc.vector.tensor_tensor(hid[:], ps_hid[:], linb_bc[:], ADD)
                nc.scalar.activation(hid[:], hid[:], AF.Relu)
                hw = wkd.tile([BL, H2], F32, tag="hw")
                nc.vector.tensor_tensor(hw[:], hid[:], outw_bc[:], MUL)
                y_sb = wkd.tile([BL, 1], F32, tag="ysb")
                nc.vector.reduce_sum(y_sb[:], hw[:], axis=AX.X)
                nc.vector.tensor_tensor(y_sb[:], y_sb[:], outb_col[:], ADD)
                nc.sync.dma_start(y.ap(), y_sb[:])

    nc.compile()
    return nc


def make_in_maps(inputs, n_cores: int, BL: int):
    """Host-side prep: shard + transpose + pad + reorder weights."""
    f32 = np.float32
    bf16 = ml_dtypes.bfloat16

    def pad_rows(a, n):
        return np.pad(a, ((0, n - a.shape[0]),) + ((0, 0),) * (a.ndim - 1))

    conv_w = np.asarray(inputs["conv_w"], f32)
    wcol = np.ascontiguousarray(
        conv_w.transpose(1, 2, 0).reshape(KK, CO)).astype(bf16)
    convb = np.ascontiguousarray(
        pad_rows(np.asarray(inputs["conv_b"], f32), 384).reshape(3, 128).T)

    # gates: blocks [i_f,i_b | g_f,g_b | o_f,o_b], K padded 320->384
    def gate_cols(w_ih, b_ih, b_hh):
        W = np.asarray(w_ih, f32).T        # [320, 640]; cols i,f,g,o per 160
        b = np.asarray(b_ih, f32) + np.asarray(b_hh, f32)
        return W, b

    Wf, bf_ = gate_cols(inputs["w_ih_f"], inputs["b_ih_f"], inputs["b_hh_f"])
    Wb, bb_ = gate_cols(inputs["w_ih_b"], inputs["b_ih_b"], inputs["b_hh_b"])
    wg2 = np.concatenate([Wf[:, 0:160], Wb[:, 0:160],
                          Wf[:, 320:480], Wb[:, 320:480],
                          Wf[:, 480:640], Wb[:, 480:640]], axis=1)  # [320,960]
    wg2 = pad_rows(wg2, 384).reshape(3, 128, 960).transpose(1, 0, 2)
    wg2 = np.ascontiguousarray(wg2).astype(bf16)
    bcat = np.concatenate([bf_[0:160], bb_[0:160], bf_[320:480],
                           bb_[320:480], bf_[480:640], bb_[480:640]])  # [960]
    gbias2 = np.zeros((128, 9), f32)
    for mk in range(9):
        w = MK_W[mk]
        gbias2[0:w, mk] = bcat[MK_OFF[mk]:MK_OFF[mk] + w]

    gene_w = np.asarray(inputs["gene_w"], f32)  # [320, 19795]
    gwT = np.ascontiguousarray(
        pad_rows(np.ascontiguousarray(gene_w.T), KG)
        .reshape(KGC, 128, CO).transpose(1, 0, 2)).astype(bf16)
    gamma = pad_rows(np.asarray(inputs["bn_gamma"], f32), 384).reshape(3, 128).T
    beta = pad_rows(np.asarray(inputs["bn_beta"], f32), 384).reshape(3, 128).T
    gbcols = np.ascontiguousarray(np.concatenate([gamma, beta], 1))

    # lin_w: [925, 14400] cols are (t, d); three d-chunk tensors [dn, T, H2]
    lin_w = np.asarray(inputs["lin_w"], f32)
    lwT = np.ascontiguousarray(lin_w.T).reshape(T, CO, H2)  # [t, d, j]
    lw0 = np.ascontiguousarray(lwT[:, 0:128].transpose(1, 0, 2)).astype(bf16)
    lw1 = np.ascontiguousarray(lwT[:, 128:256].transpose(1, 0, 2)).astype(bf16)
    lw2 = np.ascontiguousarray(lwT[:, 256:320].transpose(1, 0, 2)).astype(bf16)
    linb = np.asarray(inputs["lin_b"], f32).reshape(1, H2)
    outw = np.asarray(inputs["out_w"], f32).reshape(1, H2)
    outb = np.asarray(inputs["out_b"], f32).reshape(1, 1)

    x = np.asarray(inputs["x"], f32)
    ge = np.asarray(inputs["geneexpr"], f32)

    shared = dict(wcol=wcol, convb=convb, wg2=wg2, gbias2=gbias2, gwT=gwT,
                  gbcols=gbcols, lw0=lw0, lw1=lw1, lw2=lw2, linb=linb,
                  outw=outw, outb=outb)
    in_maps = []
    for i in range(n_cores):
        sl = slice(i * BL, (i + 1) * BL)
        geTn = np.ascontiguousarray(
            pad_rows(np.ascontiguousarray(ge[sl].T), KG)
            .reshape(KGC, 128, BL).transpose(1, 0, 2)).astype(bf16)
        m = dict(shared)
        m["x_l"] = np.ascontiguousarray(x[sl]).astype(bf16)
        m["geT"] = geTn
        in_maps.append(m)
    return in_maps


_NC_CACHE = {}


def _get_nc(n_cores, BL):
    key = (n_cores, BL)
    if key not in _NC_CACHE:
        _NC_CACHE[key] = build_nc(n_cores, BL)
    return _NC_CACHE[key]


def kernel(**inputs) -> np.ndarray:
    BL = B_TOTAL // N_CORES
    nc = _get_nc(N_CORES, BL)
    in_maps = make_in_maps(inputs, N_CORES, BL)
    res = run_bass_kernel_spmd(nc, in_maps, list(range(N_CORES)))
    return np.concatenate([res.results[i]["y"] for i in range(N_CORES)], axis=0)
